# revision 3
# baseline (speedup 1.0000x reference)
"""Self-contained Trainium2 (Bass/Tile) kernel for nn_FSUConv2d.

Math (see reference):
  ib1 = unfold(x)                             # [B, CKK] bits
  wbit1 = (w_bin > rng[i1 % 256])             # [B, OC, CKK]
  wbit0 = 1 - (w_bin > rng[i0 % 256])
  obin  = einsum('bk,bok->bo', ib1, wbit1) + einsum('bk,bok->bo', 1-ib1, wbit0)
  out   = fold(obin) + (b_bin > rng[brdx % 256])

Device-side formulation (exact, integer-valued fp16 arithmetic):
  Per element the contribution is
     ib1 ? (rng[i1] < w) : (w <= rng[i0])
  rng values r are integers in [0,255].  Host packs per (b,o) a 576-long fp16
  vector V:
     V[k]      = ib1[b,k] ? r1 : 300.0            (300 = sentinel, always "no")
     V[288+k]  = ib1[b,k] ? 300.0 : 255.5 - r0
  and per (o) a 576-long threshold vector T (half-integers, exact in fp16):
     T[k]      = ceil(w[o,k]) - 0.5               ((r <  T) <=> r <  w)
     T[288+k]  = 256.5 - ceil(w[o,k])             ((255.5-r < T) <=> w <= r)
  Then obin[b,o] = sum_j (V[b,o,j] < T[o,j]) -- computed on DVE with one
  fused scalar_tensor_tensor(is_lt, accum_out) per (128-batch-tile, o).
  T is replicated to all 128 partitions once via a K=1 PE matmul broadcast.

Sharding: data-parallel over B=2048 -> 8 cores x 256 rows.
"""

import numpy as np

_N, _C, _H, _W = 8, 32, 16, 16
_OC, _KS, _PAD = 64, 3, 1
_RLEN = 256
_CKK = _C * _KS * _KS          # 288
_B = _N * _H * _W              # 2048
_NCORES = 8
_BL = _B // _NCORES            # 256 rows per core
_F = 2 * _CKK                  # 576
_SENT = 300.0

_cache = {}


def _unfold(x):
    # torch.nn.functional.unfold ordering (c, kh, kw), zero padding 1
    xp = np.pad(x, ((0, 0), (0, 0), (_PAD, _PAD), (_PAD, _PAD)))
    cols = np.stack(
        [xp[:, :, i:i + _H, j:j + _W] for i in range(_KS) for j in range(_KS)],
        axis=2,
    )  # [N, C, K*K, H, W]
    return (
        cols.reshape(_N, _CKK, _H * _W).transpose(0, 2, 1).reshape(_B, _CKK)
    )


def _build_nc(BL=_BL, OC=_OC, F=_F, chunk_o=8, nmm=512, repeats=1):
    """Build the per-core Bass program (same NEFF on all cores)."""
    from concourse import bacc, mybir
    from concourse.tile import TileContext

    dt = mybir.dt
    CF = chunk_o * F
    nchunk = OC // chunk_o
    assert OC % chunk_o == 0 and CF % nmm == 0 and BL % 128 == 0

    nc = bacc.Bacc("TRN2", target_bir_lowering=False, debug=False)
    xc = nc.dram_tensor("xc", [BL, OC * F], dt.float16, kind="ExternalInput")
    w2 = nc.dram_tensor("w2", [1, OC * F], dt.float16, kind="ExternalInput")
    bb = nc.dram_tensor("bb", [128, OC], dt.float32, kind="ExternalInput")
    ob_d = nc.dram_tensor("obin", [BL, OC], dt.float32, kind="ExternalOutput")

    with TileContext(nc) as tc:
        with (
            tc.tile_pool(name="const", bufs=1) as constp,
            tc.tile_pool(name="stage", bufs=2) as stagep,
            tc.tile_pool(name="psum", bufs=4, space="PSUM") as psump,
            tc.tile_pool(name="xt", bufs=3) as xtp,
            tc.tile_pool(name="scr", bufs=2) as scrp,
            tc.tile_pool(name="ob", bufs=2) as obp,
        ):
            ones = constp.tile([1, 128], dt.float16)
            nc.vector.memset(ones[:], 1.0)
            bbt = constp.tile([128, OC], dt.float32)
            nc.sync.dma_start(out=bbt[:], in_=bb[:, :])

            # Replicate thresholds w2 [1, OC*F] to all 128 partitions.
            w2r = constp.tile([128, OC * F], dt.float16)
            for ci in range(nchunk):
                st = stagep.tile([1, CF], dt.float16)
                nc.sync.dma_start(out=st[:], in_=w2[:, ci * CF:(ci + 1) * CF])
                for mi in range(CF // nmm):
                    ps = psump.tile([128, nmm], dt.float32)
                    nc.tensor.matmul(
                        ps[:], ones[:], st[:, mi * nmm:(mi + 1) * nmm],
                        start=True, stop=True,
                    )
                    lo = ci * CF + mi * nmm
                    nc.scalar.copy(out=w2r[:, lo:lo + nmm], in_=ps[:])

            for bt in [b for _ in range(repeats) for b in range(BL // 128)]:
                ob = obp.tile([128, OC], dt.float32)
                for ci in range(nchunk):
                    xt = xtp.tile([128, CF], dt.float16)
                    nc.sync.dma_start(
                        out=xt[:],
                        in_=xc[bt * 128:(bt + 1) * 128, ci * CF:(ci + 1) * CF],
                    )
                    for oi in range(chunk_o):
                        o = ci * chunk_o + oi
                        sc = scrp.tile([128, F], dt.float16)
                        nc.vector.scalar_tensor_tensor(
                            out=sc[:],
                            in0=xt[:, oi * F:(oi + 1) * F],
                            scalar=0.0,
                            in1=w2r[:, o * F:(o + 1) * F],
                            op0=mybir.AluOpType.add,
                            op1=mybir.AluOpType.is_lt,
                            accum_out=ob[:, o:o + 1],
                        )
                ob2 = obp.tile([128, OC], dt.float32)
                nc.vector.tensor_add(out=ob2[:], in0=ob[:], in1=bbt[:])
                nc.sync.dma_start(
                    out=ob_d[bt * 128:(bt + 1) * 128, :], in_=ob2[:]
                )
    nc.compile()
    return nc


def _get_nc():
    if "nc" not in _cache:
        _cache["nc"] = _build_nc()
    return _cache["nc"]


def _prep_inputs(x, w_bin, b_bin, rng, wrdx_i1, wrdx_i0, brdx):
    x = np.asarray(x, np.float32)
    w_bin = np.asarray(w_bin, np.float32)
    b_bin = np.asarray(b_bin, np.float32)
    rng = np.asarray(rng, np.float32)
    wrdx_i1 = np.asarray(wrdx_i1)
    wrdx_i0 = np.asarray(wrdx_i0)
    brdx = np.asarray(brdx)

    ib1 = _unfold(x)                       # [B, CKK]
    mask = (ib1 > 0.5)[:, None, :]         # [B, 1, CKK]

    r1 = rng[wrdx_i1 % _RLEN]              # [B, OC, CKK] f32
    r0 = rng[wrdx_i0 % _RLEN]

    xcv = np.empty((_B, _OC, _F), np.float16)
    np.copyto(xcv[:, :, :_CKK], np.where(mask, r1, _SENT), casting="same_kind")
    np.copyto(
        xcv[:, :, _CKK:], np.where(mask, _SENT, 255.5 - r0), casting="same_kind"
    )

    cw = np.ceil(w_bin)                    # [OC, CKK]
    w2 = np.empty((_OC, _F), np.float16)
    w2[:, :_CKK] = cw - 0.5
    w2[:, _CKK:] = 256.5 - cw

    bbit = (b_bin > rng[brdx % _RLEN]).astype(np.float32)       # [OC]
    bb_rep = np.ascontiguousarray(
        np.broadcast_to(bbit[None, :], (128, _OC)), dtype=np.float32
    )

    in_maps = []
    for c in range(_NCORES):
        in_maps.append({
            "xc": np.ascontiguousarray(
                xcv[c * _BL:(c + 1) * _BL].reshape(_BL, _OC * _F)
            ),
            "w2": np.ascontiguousarray(w2.reshape(1, _OC * _F)),
            "bb": bb_rep,
        })
    return in_maps


def kernel(x, w_bin, b_bin, rng, wrdx_i1, wrdx_i0, brdx):
    from concourse.bass_utils import run_bass_kernel_spmd

    in_maps = _prep_inputs(x, w_bin, b_bin, rng, wrdx_i1, wrdx_i0, brdx)
    nc = _get_nc()
    res = run_bass_kernel_spmd(nc, in_maps, core_ids=list(range(_NCORES)))
    obin = np.concatenate([r["obin"] for r in res.results], axis=0)  # [B, OC]
    out = (
        obin.reshape(_N, _H * _W, _OC)
        .transpose(0, 2, 1)
        .reshape(_N, _OC, _H, _W)
    )
    return np.ascontiguousarray(out, dtype=np.float32)


# revision 7
# speedup vs baseline: 2.2513x; 2.2513x over previous
"""Self-contained Trainium2 (Bass/Tile) kernel for nn_FSUConv2d.

Reference math:
  ib1 = unfold(x)                             # [B, CKK] bits
  wbit1 = (w_bin > rng[i1 % 256])             # [B, OC, CKK]
  wbit0 = 1 - (w_bin > rng[i0 % 256])
  obin  = einsum('bk,bok->bo', ib1, wbit1) + einsum('bk,bok->bo', 1-ib1, wbit0)
  out   = fold(obin) + (b_bin > rng[brdx % 256])

Per element the contribution is  ib1 ? (r1 < w) : (w <= r0), r = rng[idx]
an integer in [0,255].  With cw = ceil(w) and integer r:
  (r < w) <=> (r < cw),  (w <= r) = 1 - (r < w) = 1 - (r < cw).

Device formulation (variant C, uint16-packed):
  Host packs v[b,o,k] = r1e*256 + r0e (uint16) with masked bytes
     r1e = ib1 ? r1 : 255       r0e = ib1 ? r0 : 0
  Then for thresholds TH = 256*cw, TL = cw (per (o,k)):
     acc1[b,o] = sum_k (v < TH)            == sum_{ib=1}(r1<w) + phantom1
                 (v < 256*cw <=> r1e < cw since r0e <= 255)
     acc0[b,o] = sum_k (v mod 256 < TL)    == sum_{ib=0}(r0<w) + phantom0
     obin = acc1 - acc0 + corr[b,o]
  corr folds z0[b] (= #{ib=0}), the sentinel phantoms, and the bias bit --
  all computed exactly on host from the small tensors.  All device math is
  exact (integers in fp32 accumulation).

Device layout:
  Stream rows r = k*64 + o (k-major, o-minor), columns = b (256 per core);
  144 SBUF tiles [128, 256] uint16.  Per tile the thresholds are
  per-partition scalars -> DVE tensor_scalar (4x mode):
     bits1 = (v is_lt TH_p)            bits0 = ((v mod 256) is_lt TL_p)
  Reduction over k on PE: fixed one-hot lhsT [128, 64] (p%64==o) with +1
  entries for bits1 and a -1 copy for bits0, accumulating psum[64, 256]
  over all 288 matmuls (weight switches batched per DMA group).
  Final: out[64, 256] = psum + corr (one small TT add), DMA out.

Sharding: data-parallel over B=2048 -> 8 cores x 256 rows (= 1 image each).
"""

import numpy as np

_N, _C, _H, _W = 8, 32, 16, 16
_OC, _KS, _PAD = 64, 3, 1
_RLEN = 256
_CKK = _C * _KS * _KS          # 288
_B = _N * _H * _W              # 2048
_NCORES = 8
_BL = _B // _NCORES            # 256 rows per core
_NROW = _CKK * _OC             # 18432 stream rows per core
_NT = _NROW // 128             # 144 tiles

_cache = {}


def _unfold(x):
    # torch.nn.functional.unfold ordering (c, kh, kw), zero padding 1
    xp = np.pad(x, ((0, 0), (0, 0), (_PAD, _PAD), (_PAD, _PAD)))
    cols = np.stack(
        [xp[:, :, i:i + _H, j:j + _W] for i in range(_KS) for j in range(_KS)],
        axis=2,
    )  # [N, C, K*K, H, W]
    return (
        cols.reshape(_N, _CKK, _H * _W).transpose(0, 2, 1).reshape(_B, _CKK)
    )


def _build_nc(BL=_BL, OC=_OC, CKK=_CKK, tgroup=16, repeats=1, loop_n=None):
    """Build the per-core Bass program (same NEFF on all cores).

    Inputs: xs [CKK*OC, BL] uint16 (rows r = k*OC + o), thrs [128, 2*NT] f32
    (columns 2t / 2t+1 = TH / TL for tile t), lhst [128, 2*OC] fp16
    (+one-hot | -one-hot), corr [OC, BL] f32.  Output: out [OC, BL] f32.
    """
    from concourse import bacc, mybir
    from concourse.tile import TileContext

    dt = mybir.dt
    NROW = CKK * OC
    NT = NROW // 128
    assert NROW % 128 == 0 and NT % tgroup == 0 and 128 % OC == 0

    nc = bacc.Bacc("TRN2", target_bir_lowering=False, debug=False)
    xs = nc.dram_tensor("xs", [NROW, BL], dt.uint16, kind="ExternalInput")
    th_d = nc.dram_tensor("thrs", [128, 2 * NT], dt.float32, kind="ExternalInput")
    lh_d = nc.dram_tensor("lhst", [128, 2 * OC], dt.float16, kind="ExternalInput")
    co_d = nc.dram_tensor("corr", [OC, BL], dt.float32, kind="ExternalInput")
    out_d = nc.dram_tensor("out", [OC, BL], dt.float32, kind="ExternalOutput")

    with TileContext(nc) as tc:
        with (
            tc.tile_pool(name="const", bufs=1) as constp,
            tc.tile_pool(name="xt", bufs=3) as xtp,
            tc.tile_pool(name="bits", bufs=2 * tgroup + 4) as bitsp,
            tc.tile_pool(name="psum", bufs=2, space="PSUM") as psump,
            tc.tile_pool(name="outp", bufs=2) as outp,
        ):
            thrs = constp.tile([128, 2 * NT], dt.float32)
            nc.sync.dma_start(out=thrs[:], in_=th_d[:, :])
            lhst = constp.tile([128, 2 * OC], dt.float16)
            nc.sync.dma_start(out=lhst[:], in_=lh_d[:, :])
            corr = constp.tile([OC, BL], dt.float32)
            nc.sync.dma_start(out=corr[:], in_=co_d[:, :])

            def body():
                ps = psump.tile([OC, BL], dt.float32)
                for g in range(NT // tgroup):
                    xt = xtp.tile([128, tgroup, BL], dt.uint16)
                    src = xs[g * tgroup * 128:(g + 1) * tgroup * 128, :]
                    nc.sync.dma_start(
                        out=xt[:], in_=src.rearrange("(t p) b -> p t b", p=128)
                    )
                    b1s, b0s = [], []
                    for ti in range(tgroup):
                        t = g * tgroup + ti
                        b1 = bitsp.tile([128, BL], dt.float16, tag="bits")
                        nc.vector.tensor_scalar(
                            out=b1[:], in0=xt[:, ti, :],
                            scalar1=thrs[:, 2 * t:2 * t + 1], scalar2=None,
                            op0=mybir.AluOpType.is_lt,
                        )
                        b1s.append(b1)
                        b0 = bitsp.tile([128, BL], dt.float16, tag="bits")
                        nc.vector.tensor_scalar(
                            out=b0[:], in0=xt[:, ti, :],
                            scalar1=256.0,
                            scalar2=thrs[:, 2 * t + 1:2 * t + 2],
                            op0=mybir.AluOpType.mod,
                            op1=mybir.AluOpType.is_lt,
                        )
                        b0s.append(b0)
                    # batch matmuls by stationary weight: first all +, then all -
                    for ti in range(tgroup):
                        nc.tensor.matmul(
                            ps[:], lhst[:, :OC], b1s[ti][:],
                            start=(g == 0 and ti == 0), stop=False,
                        )
                    for ti in range(tgroup):
                        nc.tensor.matmul(
                            ps[:], lhst[:, OC:], b0s[ti][:],
                            start=False,
                            stop=(g == NT // tgroup - 1 and ti == tgroup - 1),
                        )
                ot = outp.tile([OC, BL], dt.float32)
                nc.vector.tensor_tensor(
                    out=ot[:], in0=ps[:], in1=corr[:], op=mybir.AluOpType.add
                )
                nc.sync.dma_start(out=out_d[:, :], in_=ot[:])

            if loop_n is not None:
                with tc.For_i(0, loop_n, 1):
                    body()
            else:
                for _ in range(repeats):
                    body()
    nc.compile()
    return nc


def _get_nc():
    if "nc" not in _cache:
        _cache["nc"] = _build_nc()
    return _cache["nc"]


def _prep_inputs(x, w_bin, b_bin, rng, wrdx_i1, wrdx_i0, brdx):
    x = np.asarray(x, np.float32)
    w_bin = np.asarray(w_bin, np.float32)
    b_bin = np.asarray(b_bin, np.float32)
    rng = np.asarray(rng, np.float32)
    wrdx_i1 = np.asarray(wrdx_i1)
    wrdx_i0 = np.asarray(wrdx_i0)
    brdx = np.asarray(brdx)

    ib1 = _unfold(x)                       # [B, CKK] {0,1}
    mask = (ib1 > 0.5)[:, None, :]         # [B, 1, CKK]

    rng_i = np.rint(rng).astype(np.int32)
    # device scheme needs integer rng values in [0, 255] (true for the
    # reference Sobol table and for arange fills)
    assert np.all(np.abs(rng - rng_i) < 1e-6) and rng_i.min() >= 0 \
        and rng_i.max() <= 255, "rng must be integers in [0,255]"

    r1 = rng_i[wrdx_i1 % _RLEN]            # [B, OC, CKK] int32
    r0 = rng_i[wrdx_i0 % _RLEN]

    v = np.where(mask, r1, 255).astype(np.uint32) << 8
    v |= np.where(mask, r0, 0).astype(np.uint32)
    v = v.astype(np.uint16)                # [B, OC, CKK]

    cw = np.ceil(w_bin)                    # [OC, CKK] in [0, 256]

    # thresholds per stream row r = k*OC + o, tiled [128, NT] -> interleaved
    TH = (256.0 * cw).T.reshape(_NT, 128).T    # [128, NT] row-major (k,o)
    TL = cw.T.reshape(_NT, 128).T
    thrs = np.empty((128, 2 * _NT), np.float32)
    thrs[:, 0::2] = TH
    thrs[:, 1::2] = TL

    onehot = (
        np.arange(128)[:, None] % _OC == np.arange(_OC)[None, :]
    ).astype(np.float16)
    lhst = np.concatenate([onehot, -onehot], axis=1)   # [128, 2*OC]

    # corrections: obin = acc1 - acc0 + corr
    ibf = ib1.astype(np.float32)                       # [B, CKK]
    z0 = (_CKK - ibf.sum(axis=1))[:, None]             # [B, 1]
    phantom1 = (1.0 - ibf) @ (cw == 256.0).astype(np.float32).T  # [B, OC]
    phantom0 = ibf @ (cw >= 1.0).astype(np.float32).T            # [B, OC]
    bbit = (b_bin > rng[brdx % _RLEN]).astype(np.float32)        # [OC]
    corr_bo = z0 + phantom0 - phantom1 + bbit[None, :]           # [B, OC]

    in_maps = []
    for c in range(_NCORES):
        vc = v[c * _BL:(c + 1) * _BL]                  # [BL, OC, CKK]
        xsrc = np.ascontiguousarray(
            vc.transpose(2, 1, 0).reshape(_NROW, _BL)
        )  # rows r = k*OC + o
        in_maps.append({
            "xs": xsrc,
            "thrs": thrs,
            "lhst": lhst,
            "corr": np.ascontiguousarray(
                corr_bo[c * _BL:(c + 1) * _BL].T, dtype=np.float32
            ),
        })
    return in_maps


def kernel(x, w_bin, b_bin, rng, wrdx_i1, wrdx_i0, brdx):
    from concourse.bass_utils import run_bass_kernel_spmd

    in_maps = _prep_inputs(x, w_bin, b_bin, rng, wrdx_i1, wrdx_i0, brdx)
    nc = _get_nc()
    res = run_bass_kernel_spmd(nc, in_maps, core_ids=list(range(_NCORES)))
    # out[c] is [OC, BL=H*W] for image n=c  ->  [N, OC, H, W]
    out = np.stack([r["out"] for r in res.results], axis=0)
    return np.ascontiguousarray(
        out.reshape(_N, _OC, _H, _W), dtype=np.float32
    )
